# revision 7
# baseline (speedup 1.0000x reference)
"""Trainium2 Bass kernel for a 3-layer LSTM (input=1, hidden=32) + FC head.

Problem: x (32,2,32,32,64) -> N=65536 sequences of length T=64, input size 1.
3 stacked LSTM layers (H=32, PyTorch gate order i,f,g,o), FC(32->1) on the
last hidden state of layer 2. Output (32,2,32,32).

Sharding: pure data parallel, 8192 sequences per NeuronCore across 8 cores.

Per-core layout ("stacked subtiles"):
  - 8192 seqs = 4 streams x 4 subtiles x 512 seqs.
  - State tiles h/c are [128, 512]: partition block 32j..32j+31 holds hidden
    units of subtile j; free dim is the 512 seqs. One tile per (layer, stream).
  - Gate pre-activations: one PSUM bank [128, 512] per gate, written by 16
    small matmuls on the diagonal 32x32 PE array tiles (tile_position=(32j,32j)):
    rec: W_hh^T (K=32) + inp: W_ih^T (K=32, layer>0) or x row-select (K=4,
    layer 0), accumulating in PSUM.
  - Layers run as a wavefront: at wall-step tau, layer l processes t = tau-l,
    so layer l+1 consumes h^l produced one wall-step earlier. Emission is in
    descending l so Tile's WAR deps give layer l+1 the old h^l.
  - Biases are folded into the ScalarE activation bias operand (per-partition).
"""

import numpy as np

B, C, HS, WS = 32, 2, 32, 32
T = 64
H = 32
NCORES = 8
NSEQ = B * C * HS * WS          # 65536
NPC = NSEQ // NCORES            # 8192 per core
NSTREAM = 4                     # streams per core
NSUB = 4                        # subtiles per stream (partition blocks)
FD = 512                        # seqs per subtile (free dim)
TC = 8                          # x time-chunk size
MM_DT = "bf16"                  # matmul operand dtype: "f32" | "bf16"

_CACHE = {}


def _build_bass():
    NCH = T // TC
    import sys
    if '/opt/trn_rl_repo' not in sys.path:
        sys.path.insert(0, '/opt/trn_rl_repo')
    import concourse.bacc as bacc
    import concourse.mybir as mybir
    from concourse.tile import TileContext

    F32 = mybir.dt.float32
    AF = mybir.ActivationFunctionType
    OP = mybir.AluOpType

    MMD = mybir.dt.bfloat16 if MM_DT == "bf16" else F32

    nc = bacc.Bacc("TRN2", target_bir_lowering=False, debug=False)

    xin = nc.declare_dram_parameter("xin", [NCH, NSUB, NSTREAM, TC, FD], MMD, isOutput=False)
    wts = nc.declare_dram_parameter("wts", [128, 9 * 128], MMD, isOutput=False)
    bia = nc.declare_dram_parameter("bia", [128, 12], F32, isOutput=False)
    fcw = nc.declare_dram_parameter("fcw", [128, 1], MMD, isOutput=False)
    fcb = nc.declare_dram_parameter("fcb", [128, 1], F32, isOutput=False)
    y = nc.declare_dram_parameter("y", [NSTREAM, NSUB, FD], F32, isOutput=True)

    with TileContext(nc) as tc:
        with (
            tc.sbuf_pool(name="per", bufs=1) as per,
            tc.sbuf_pool(name="trans", bufs=3) as trans,
            tc.psum_pool(name="ps", bufs=8) as ps,
        ):
            wts_sb = per.tile([128, 9 * 128], MMD)
            bia_sb = per.tile([128, 12], F32)
            fcw_sb = per.tile([128, 1], MMD)
            fcb_sb = per.tile([128, 1], F32)
            nc.sync.dma_start(out=wts_sb[:], in_=wts[:])
            nc.sync.dma_start(out=bia_sb[:], in_=bia[:])
            nc.sync.dma_start(out=fcw_sb[:], in_=fcw[:])
            nc.sync.dma_start(out=fcb_sb[:], in_=fcb[:])

            # persistent state tiles
            h_t = [[per.tile([128, FD], MMD, name=f"h_{l}_{s}", tag=f"h_{l}_{s}")
                    for s in range(NSTREAM)] for l in range(3)]
            c_t = [[per.tile([128, FD], F32, name=f"c_{l}_{s}", tag=f"c_{l}_{s}")
                    for s in range(NSTREAM)] for l in range(3)]
            xt = [per.tile([128, TC * FD], MMD, name=f"xt{i}", tag=f"xt{i}") for i in range(2)]
            y_sb = per.tile([128, NSTREAM * FD], F32)

            def load_chunk(k):
                for j in range(NSUB):
                    nc.sync.dma_start(
                        out=xt[k % 2][32 * j:32 * j + NSTREAM, :],
                        in_=xin[k, j].rearrange("s tc n -> s (tc n)"),
                    )

            load_chunk(0)

            def step(l, s, t):
                gates = [ps.tile([128, FD], F32, name=f"g{l}_{s}_{t}_{g}", tag="gate")
                         for g in range(4)]
                hl = h_t[l][s]
                for g in range(4):
                    for j in range(4):
                        pj = slice(32 * j, 32 * j + 32)
                        tp = (32 * j, 32 * j)
                        if t > 0:
                            nc.tensor.matmul(
                                gates[g][pj, :],
                                (wts_sb[pj, l * 128 + 32 * g: l * 128 + 32 * g + 32]),
                                (hl[pj, :]),
                                start=True, stop=False, tile_position=tp,
                            )
                        if l == 0:
                            k = t // TC
                            off = (t % TC) * FD
                            nc.tensor.matmul(
                                gates[g][pj, :],
                                (wts_sb[32 * j:32 * j + NSTREAM,
                                           (5 + s) * 128 + 32 * g: (5 + s) * 128 + 32 * g + 32]),
                                (xt[k % 2][32 * j:32 * j + NSTREAM, off:off + FD]),
                                start=(t == 0), stop=True, tile_position=tp,
                            )
                        else:
                            nc.tensor.matmul(
                                gates[g][pj, :],
                                (wts_sb[pj, (2 + l) * 128 + 32 * g: (2 + l) * 128 + 32 * g + 32]),
                                (h_t[l - 1][s][pj, :]),
                                start=(t == 0), stop=True, tile_position=tp,
                            )

                def bap(g):
                    return bia_sb[:, l * 4 + g: l * 4 + g + 1]

                sig_i = trans.tile([128, FD], MMD, name=f"si{l}_{s}_{t}", tag="sig_i")
                tan_g = trans.tile([128, FD], MMD, name=f"tg{l}_{s}_{t}", tag="tan_g")
                sig_o = trans.tile([128, FD], MMD, name=f"so{l}_{s}_{t}", tag="sig_o")
                nc.scalar.activation(sig_i[:], gates[0][:], AF.Sigmoid, bias=bap(0))
                if t > 0:
                    sig_f = trans.tile([128, FD], F32, name=f"sf{l}_{s}_{t}", tag="sig_f")
                    nc.scalar.activation(sig_f[:], gates[1][:], AF.Sigmoid, bias=bap(1))
                nc.scalar.activation(tan_g[:], gates[2][:], AF.Tanh, bias=bap(2))
                nc.scalar.activation(sig_o[:], gates[3][:], AF.Sigmoid, bias=bap(3))

                ct = c_t[l][s]
                if t == 0:
                    nc.vector.tensor_tensor(ct[:], sig_i[:], tan_g[:], OP.mult)
                else:
                    tmp = trans.tile([128, FD], MMD, name=f"tm{l}_{s}_{t}", tag="tmp")
                    nc.vector.tensor_tensor(tmp[:], sig_i[:], tan_g[:], OP.mult)
                    nc.vector.tensor_tensor(ct[:], sig_f[:], ct[:], OP.mult)
                    nc.vector.tensor_tensor(ct[:], ct[:], tmp[:], OP.add)
                tan_c = trans.tile([128, FD], MMD, name=f"tc{l}_{s}_{t}", tag="tan_c")
                nc.scalar.activation(tan_c[:], ct[:], AF.Tanh)
                nc.vector.tensor_tensor(hl[:], sig_o[:], tan_c[:], OP.mult)

            def fc(s):
                pfc = ps.tile([128, FD], F32, name=f"pfc{s}", tag="gate")
                for j in range(4):
                    pj = slice(32 * j, 32 * j + 32)
                    nc.tensor.matmul(
                        pfc[32 * j:32 * j + 1, :], (fcw_sb[pj, 0:1]), (h_t[2][s][pj, :]),
                        start=True, stop=True, tile_position=(32 * j, 32 * j),
                    )
                for j in range(4):
                    r = slice(32 * j, 32 * j + 1)
                    nc.scalar.activation(
                        y_sb[r, s * FD:(s + 1) * FD], pfc[r, :], AF.Identity,
                        bias=fcb_sb[r, :],
                    )
                for j in range(4):
                    nc.sync.dma_start(
                        out=y[s, j:j + 1, :],
                        in_=y_sb[32 * j:32 * j + 1, s * FD:(s + 1) * FD],
                    )

            for tau in range(T + 2):
                if tau % TC == TC // 2 and tau // TC + 1 < NCH:
                    load_chunk(tau // TC + 1)
                for l in (2, 1, 0):
                    t = tau - l
                    if not (0 <= t < T):
                        continue
                    for s in range(NSTREAM):
                        step(l, s, t)
                        if l == 2 and t == T - 1:
                            fc(s)

    nc.compile()
    return nc


def _np_mmd():
    if MM_DT == "bf16":
        import ml_dtypes
        return ml_dtypes.bfloat16
    return np.float32


def _prep_inputs(x, w_ih0, w_hh0, b_ih0, b_hh0, w_ih1, w_hh1, b_ih1, b_hh1,
                 w_ih2, w_hh2, b_ih2, b_hh2, fc_w, fc_b):
    NCH = T // TC
    x_flat = np.ascontiguousarray(x, dtype=np.float32).reshape(NSEQ, T)
    w_hh = [w_hh0, w_hh1, w_hh2]
    w_ih = [w_ih0, w_ih1, w_ih2]
    b_sum = [b_ih0 + b_hh0, b_ih1 + b_hh1, b_ih2 + b_hh2]

    wts = np.zeros((9, 128, 128), np.float32)  # packed to [128, 9*128] below
    for l in range(3):
        # wts[l][32j+k, 32g+u] = w_hh_l[32g+u, k]
        blk = np.asarray(w_hh[l], np.float32).reshape(128, 32).T  # [k, 128]
        for j in range(4):
            wts[l, 32 * j:32 * j + 32, :] = blk
    for l in (1, 2):
        blk = np.asarray(w_ih[l], np.float32).reshape(128, 32).T
        for j in range(4):
            wts[2 + l, 32 * j:32 * j + 32, :] = blk
    w0 = np.asarray(w_ih0, np.float32).reshape(128)  # input size 1
    for s in range(4):
        for j in range(4):
            wts[5 + s, 32 * j + s, :] = w0

    bia = np.zeros((128, 12), np.float32)
    for l in range(3):
        bb = np.asarray(b_sum[l], np.float32).reshape(4, 32)  # [g, u]
        for g in range(4):
            for j in range(4):
                bia[32 * j:32 * j + 32, l * 4 + g] = bb[g]

    fcw = np.zeros((128, 1), np.float32)
    fw = np.asarray(fc_w, np.float32).reshape(32)
    for j in range(4):
        fcw[32 * j:32 * j + 32, 0] = fw
    fcb = np.full((128, 1), np.float32(np.asarray(fc_b).reshape(())), np.float32)
    wts_packed = np.ascontiguousarray(wts.transpose(1, 0, 2).reshape(128, 9 * 128))

    mmd = _np_mmd()
    wts_packed = wts_packed.astype(mmd)
    fcw = fcw.astype(mmd)
    in_maps = []
    for core in range(NCORES):
        xc = x_flat[core * NPC:(core + 1) * NPC]  # [8192, 64]
        # xin[k, j, s, tc, n] = xc[s*2048 + j*512 + n, k*TC + tc]
        xv = xc.reshape(NSTREAM, NSUB, FD, NCH, TC)
        xin = np.ascontiguousarray(xv.transpose(3, 1, 0, 4, 2)).astype(mmd)
        in_maps.append({"xin": xin, "wts": wts_packed, "bia": bia, "fcw": fcw, "fcb": fcb})
    return in_maps


def _run(in_maps, trace=False):
    import sys
    if '/opt/trn_rl_repo' not in sys.path:
        sys.path.insert(0, '/opt/trn_rl_repo')
    from concourse.bass_utils import run_bass_kernel_spmd
    if "nc" not in _CACHE:
        _CACHE["nc"] = _build_bass()
    nc = _CACHE["nc"]
    res = run_bass_kernel_spmd(nc, in_maps, list(range(NCORES)), trace=trace)
    return res


def kernel(**inputs):
    in_maps = _prep_inputs(**inputs)
    res = _run(in_maps)
    outs = []
    for core in range(NCORES):
        yc = res.results[core]["y"]  # [s, j, 512]
        outs.append(yc.reshape(NSTREAM, NSUB, FD).reshape(NPC))
    full = np.concatenate(outs)  # [65536]
    return full.reshape(B, C, HS, WS).astype(np.float32)
